# revision 1
# baseline (speedup 1.0000x reference)
"""GAT node encoder (3 GATConv+BN layers) on 8 trn2 NeuronCores.

Sharding: nodes partitioned across cores (dst-sharded message passing).
Per layer, per core:
  1. local matmul of this core's node shard: [h | s | d] = y @ [W | W@a_src | W@a_dst]
  2. AllGather of the [h | s] node table (node-major rows) across cores
  3. per dst-tile (128 nodes, degree-sorted ELL layout): indirect-DMA row
     gathers of h[src], flash-style segment softmax over incoming edges,
     weighted accumulation, head mean
  4. BatchNorm: feature-major stats via free-axis reduction + AllReduce of
     per-feature sums, fused scale/shift(+ReLU) activation.

The per-feature bias b is dropped: BN(o + b) == BN(o) exactly (b shifts every
node's feature equally, so it cancels in mean subtraction and leaves var
unchanged).
"""
import os
import sys

sys.path.insert(0, "/opt/trn_rl_repo")

import numpy as np

import concourse.bass as bass
import concourse.bacc as bacc
import concourse.tile as tile
from concourse import mybir
from concourse import bass_utils
from concourse.masks import make_identity

NCORES = 8
P = 128
NEG_SLOPE = 0.2
EPS_BN = 1e-5
CHUNK = 12  # ELL slots processed per flash-softmax chunk

F32 = mybir.dt.float32
I32 = mybir.dt.int32


# ----------------------------------------------------------------------------
# host-side graph preprocessing
# ----------------------------------------------------------------------------

def _prep(edge_index, N):
    src = np.asarray(edge_index[0], dtype=np.int64)
    dst = np.asarray(edge_index[1], dtype=np.int64)
    loops = np.arange(N, dtype=np.int64)
    src = np.concatenate([src, loops])
    dst = np.concatenate([dst, loops])

    shard = N // NCORES                      # real nodes per core
    ntiles = (shard + P) // P                # always >= 1 pad row per shard
    shard_pad = ntiles * P                   # padded rows per core shard
    pad_row = shard                          # global table row of a guaranteed pad node (rank 0)

    # per-core node permutation (degree-descending) + global row ids
    deg = np.bincount(dst, minlength=N)
    node_row = np.empty(N, np.int64)         # orig node -> global table row
    core_nodes = []                          # core -> orig node id per local row (len shard_pad, -1 pad)
    for c in range(NCORES):
        lo = c * shard
        nodes = np.arange(lo, lo + shard)
        order = np.argsort(-deg[lo:lo + shard], kind="stable")
        nodes = nodes[order]
        node_row[nodes] = c * shard_pad + np.arange(shard)
        padded = np.full(shard_pad, -1, np.int64)
        padded[:shard] = nodes
        core_nodes.append(padded)

    # per-tile slot widths S_t (max over cores so the SPMD program is uniform)
    # and per-core slot index arrays
    S = np.zeros(ntiles, np.int64)
    per_core = []
    for c in range(NCORES):
        nodes = core_nodes[c]
        degs = np.where(nodes >= 0, deg[np.maximum(nodes, 0)], 0)
        S = np.maximum(S, degs.reshape(ntiles, P).max(axis=1))
        per_core.append(degs)
    S = np.maximum(S, 1)

    offs = np.zeros(ntiles + 1, np.int64)
    offs[1:] = np.cumsum(S)
    stot = int(offs[-1])

    # build slot index arrays: idx[core][p, off_t + j] = table row of src (pad_row for empty)
    idx = np.full((NCORES, P, stot), pad_row, np.int32)
    # bucket edges by (core, local dst row)
    local = node_row[dst]                    # dst's global row
    c_of = local // shard_pad
    r_of = local % shard_pad
    order = np.lexsort((r_of, c_of))
    src_s, c_s, r_s = node_row[src][order], c_of[order], r_of[order]
    # within each (c, r) run, slot j = running position
    boundaries = np.flatnonzero(np.r_[True, (c_s[1:] != c_s[:-1]) | (r_s[1:] != r_s[:-1])])
    run_id = np.zeros(len(c_s), np.int64)
    run_id[boundaries] = 1
    run_id = np.cumsum(run_id) - 1
    j_in_run = np.arange(len(c_s)) - boundaries[run_id]
    t_s = r_s // P
    p_s = r_s % P
    idx[c_s, p_s, offs[t_s] + j_in_run] = src_s.astype(np.int32)

    out_of_core = [core_nodes[c][:shard] for c in range(NCORES)]  # orig node per local row
    return {
        "shard": shard, "shard_pad": shard_pad, "ntiles": ntiles,
        "S": S.astype(int).tolist(), "offs": offs.astype(int).tolist(),
        "stot": stot, "idx": idx, "node_row": node_row,
        "out_nodes": out_of_core, "pad_row": pad_row,
    }


# ----------------------------------------------------------------------------
# device program
# ----------------------------------------------------------------------------

def _build_program(g, layers, in_dim, ablate=()):
    """layers: list of dicts {H, C, R, hs_off} per layer.
    R = table row f32 elems (h | s | pad), hs_off = offset of s in row (= H*C).
    ablate: subset of {"gather", "edge", "coll", "mm"} to skip (timing studies).
    """
    ablate = set(ablate)
    shard_pad, ntiles = g["shard_pad"], g["ntiles"]
    S, offs, stot = g["S"], g["offs"], g["stot"]
    nrows = NCORES * shard_pad
    n_l = len(layers)

    nc = bacc.Bacc("TRN2", target_bir_lowering=False, debug=False, num_devices=NCORES)

    xT = nc.dram_tensor("xT", [in_dim, shard_pad], F32, kind="ExternalInput").ap()
    idx_in = nc.dram_tensor("idx", [P, stot], I32, kind="ExternalInput").ap()
    wexts = [nc.dram_tensor(f"wext{l}", [(in_dim if l == 0 else layers[l - 1]["C"]),
                                         layers[l]["H"] * layers[l]["C"] + 2 * layers[l]["H"]],
                            F32, kind="ExternalInput").ap() for l in range(n_l)]
    gb = nc.dram_tensor("gb", [P, 2 * n_l], F32, kind="ExternalInput").ap()  # gamma|beta columns per layer
    spad_in = nc.dram_tensor("spad", [P, 1], F32, kind="ExternalInput").ap()  # 0 / -1e30 pad-row column
    out_t = nc.dram_tensor("out", [P, shard_pad], F32, kind="ExternalOutput").ap()

    with tile.TileContext(nc) as tc:
        import contextlib
        with contextlib.ExitStack() as ctx:
            dram = ctx.enter_context(tc.tile_pool(name="dram", bufs=1, space="DRAM"))
            psum = ctx.enter_context(tc.tile_pool(name="psum", bufs=2, space="PSUM"))
            psum4 = ctx.enter_context(tc.tile_pool(name="psum4", bufs=4, space="PSUM"))
            sb = ctx.enter_context(tc.tile_pool(name="sb", bufs=1))
            sb2 = ctx.enter_context(tc.tile_pool(name="sb2", bufs=2))
            sb3 = ctx.enter_context(tc.tile_pool(name="sb3", bufs=3))
            sb4 = ctx.enter_context(tc.tile_pool(name="sb4", bufs=4))
            sb6 = ctx.enter_context(tc.tile_pool(name="sb6", bufs=4))
            sb12 = ctx.enter_context(tc.tile_pool(name="sb12", bufs=12))

            ident = sb.tile([P, P], F32, tag="ident")
            make_identity(nc, ident[:])
            idx_t = sb.tile([P, stot], I32, tag="idx")
            nc.sync.dma_start(idx_t[:], idx_in[:])
            gb_t = sb.tile([P, 2 * n_l], F32, tag="gb")
            nc.sync.dma_start(gb_t[:], gb[:])
            spad_t = sb.tile([P, 1], F32, tag="spad")
            nc.sync.dma_start(spad_t[:], spad_in[:])

            yT = None  # [P(feat), shard_pad] SBUF, input to next layer (None => xT DRAM)
            for l, L in enumerate(layers):
                H, C, R, s_off = L["H"], L["C"], L["R"], L["hs_off"]
                HC = H * C
                K = in_dim if l == 0 else layers[l - 1]["C"]
                kchunks = K // P

                ag_in = dram.tile([shard_pad, R], F32, tag=f"agin{l}")
                table = dram.tile([nrows, R], F32, tag=f"table{l}")

                wk = []
                for k in range(kchunks):
                    w = sb2.tile([P, HC + 2 * H], F32, tag="wext")
                    nc.sync.dma_start(w[:], wexts[l][k * P:(k + 1) * P, :])
                    wk.append(w)

                d_buf = sb.tile([P, ntiles * H], F32, tag=f"dbuf{l % 2}")

                # ---- phase 1: local shard matmul -> hs rows + d ----
                for t in range(ntiles):
                    ph = psum.tile([P, HC], F32, tag="mmh", space="PSUM")
                    psd = psum.tile([P, 2 * H], F32, tag="mmsd", space="PSUM")
                    for k in range(kchunks if "mm" not in ablate else 0):
                        if yT is None:
                            lhsT = sb4.tile([P, P], F32, tag="xt")
                            nc.sync.dma_start(lhsT[:], xT[k * P:(k + 1) * P, t * P:(t + 1) * P])
                            lhs_ap = lhsT[:]
                        else:
                            lhs_ap = yT[:, t * P:(t + 1) * P]
                        nc.tensor.matmul(ph[:], lhsT=lhs_ap, rhs=wk[k][:, :HC],
                                         start=(k == 0), stop=(k == kchunks - 1))
                        nc.tensor.matmul(psd[:], lhsT=lhs_ap, rhs=wk[k][:, HC:HC + 2 * H],
                                         start=(k == 0), stop=(k == kchunks - 1))
                    hs = sb3.tile([P, R], F32, tag="hs")
                    if "mmcopy" not in ablate:
                        nc.scalar.copy(hs[:, :HC], ph[:])
                        nc.vector.tensor_copy(hs[:, s_off:s_off + H], psd[:, :H])
                        if R > s_off + H:
                            nc.vector.memset(hs[:, s_off + H:], 0.0)
                        nc.vector.tensor_copy(d_buf[:, t * H:(t + 1) * H], psd[:, H:2 * H])
                    if t == ntiles - 1:
                        # pad nodes: s += -1e30 so padded slots die in the softmax
                        nc.vector.tensor_tensor(out=hs[:, s_off:s_off + H],
                                                in0=hs[:, s_off:s_off + H],
                                                in1=spad_t[:].broadcast_to([P, H]),
                                                op=mybir.AluOpType.add)
                    if "mmstore" not in ablate and "mmcopy" not in ablate:
                        nc.sync.dma_start(ag_in[t * P:(t + 1) * P, :], hs[:])

                # ---- phase 2: AllGather the node table ----
                if "coll" not in ablate:
                    nc.gpsimd.collective_compute(
                        "AllGather", mybir.AluOpType.bypass,
                        replica_groups=[list(range(NCORES))],
                        ins=[ag_in.opt()], outs=[table.opt()],
                    )

                # ---- phase 3: gather + segment softmax + weighted accumulation ----
                oT = sb.tile([P, shard_pad], F32, tag="oT")
                GRP = 4
                for g0 in range(0, ntiles, GRP):
                    gs = min(GRP, ntiles - g0)
                    accg = sb2.tile([P, GRP * HC], F32, tag="accg")
                    deng = sb12.tile([P, GRP * H], F32, tag="deng")
                    for gi in range(gs):
                        t = g0 + gi
                        st = S[t]
                        d_ap = d_buf[:, t * H:(t + 1) * H]
                        acc_slice = accg[:, gi * HC:(gi + 1) * HC]
                        den_slice = deng[:, gi * H:(gi + 1) * H]
                        acc = den = m = None
                        j0 = 0
                        while j0 < st:
                            jc = min(CHUNK, st - j0)
                            last = (j0 + jc >= st)
                            hg = sb3.tile([P, CHUNK * R], F32, tag="hg")
                            for j in range(jc if "gather" not in ablate else 0):
                                nc.gpsimd.indirect_dma_start(
                                    out=hg[:, (j * R):(j * R + R)],
                                    out_offset=None,
                                    in_=table[:],
                                    in_offset=bass.IndirectOffsetOnAxis(
                                        ap=idx_t[:, offs[t] + j0 + j: offs[t] + j0 + j + 1],
                                        axis=0),
                                )
                            if "edge" in ablate:
                                j0 += jc
                                continue
                            hg3 = hg[:].rearrange("p (j r) -> p j r", j=CHUNK)
                            e2 = sb12.tile([P, H * CHUNK], F32, tag="e2")
                            e2v = e2[:, :H * jc].rearrange("p (h j) -> p h j", h=H)
                            nc.vector.tensor_tensor(
                                out=e2v,
                                in0=hg3[:, :jc, s_off:s_off + H].transpose([0, 2, 1]),
                                in1=d_ap.unsqueeze(2).broadcast_to([P, H, jc]),
                                op=mybir.AluOpType.add)
                            nc.vector.scalar_tensor_tensor(
                                out=e2v, in0=e2v, scalar=NEG_SLOPE, in1=e2v,
                                op0=mybir.AluOpType.mult, op1=mybir.AluOpType.max)
                            mc = sb12.tile([P, H], F32, tag="mc")
                            nc.vector.tensor_reduce(out=mc[:], in_=e2v,
                                                    axis=mybir.AxisListType.X, op=mybir.AluOpType.max)
                            if m is not None:
                                mnew = sb12.tile([P, H], F32, tag="mc")
                                nc.vector.tensor_tensor(out=mnew[:], in0=m[:], in1=mc[:], op=mybir.AluOpType.max)
                                so = sb12.tile([P, H], F32, tag="so")
                                nc.vector.tensor_tensor(out=so[:], in0=m[:], in1=mnew[:], op=mybir.AluOpType.subtract)
                                nc.scalar.activation(so[:], so[:], mybir.ActivationFunctionType.Exp)
                                m = mnew
                            else:
                                m = mc
                            pb = sb12.tile([P, H * CHUNK], F32, tag="pb")
                            pbv = pb[:, :H * jc].rearrange("p (h j) -> p h j", h=H)
                            nc.vector.tensor_tensor(out=pbv, in0=e2v,
                                                    in1=m[:].unsqueeze(2).broadcast_to([P, H, jc]),
                                                    op=mybir.AluOpType.subtract)
                            nc.scalar.activation(pb[:, :H * jc], pb[:, :H * jc],
                                                 mybir.ActivationFunctionType.Exp)
                            if last and den is None:
                                ds_out = den_slice
                            else:
                                ds_tile = sb12.tile([P, H], F32, tag="ds")
                                ds_out = ds_tile[:]
                            nc.vector.tensor_reduce(out=ds_out, in_=pbv,
                                                    axis=mybir.AxisListType.X, op=mybir.AluOpType.add)
                            if den is not None:
                                if last:
                                    dn = den_slice
                                else:
                                    dn_tile = sb12.tile([P, H], F32, tag="ds")
                                    dn = dn_tile[:]
                                nc.vector.tensor_tensor(out=dn, in0=den, in1=so[:], op=mybir.AluOpType.mult)
                                nc.vector.tensor_tensor(out=dn, in0=dn, in1=ds_out, op=mybir.AluOpType.add)
                                den = dn
                            else:
                                den = ds_out
                            hgw = sb3.tile([P, CHUNK * R], F32, tag="hg")
                            nc.vector.tensor_tensor(
                                out=hgw[:, :jc * HC].rearrange("p (j h c) -> p j h c", j=jc, h=H),
                                in0=hg3[:, :jc, :HC].rearrange("p j (h c) -> p j h c", h=H),
                                in1=pb[:, :H * jc].rearrange("p (h j) -> p h j", h=H)
                                    .transpose([0, 2, 1]).unsqueeze(3).broadcast_to([P, jc, H, C]),
                                op=mybir.AluOpType.mult)
                            if last and acc is None:
                                red_out = acc_slice
                            else:
                                red_tile = sb6.tile([P, HC], F32, tag="acc")
                                red_out = red_tile[:]
                            nc.vector.tensor_tensor(out=red_out, in0=hgw[:, :HC],
                                                    in1=hgw[:, HC:2 * HC] if jc > 1 else hgw[:, :HC],
                                                    op=mybir.AluOpType.add if jc > 1 else mybir.AluOpType.bypass)
                            for jj in range(2, jc):
                                nc.vector.tensor_tensor(out=red_out, in0=red_out,
                                                        in1=hgw[:, jj * HC:(jj + 1) * HC],
                                                        op=mybir.AluOpType.add)
                            if acc is not None:
                                if last:
                                    an = acc_slice
                                else:
                                    an_tile = sb6.tile([P, HC], F32, tag="acc")
                                    an = an_tile[:]
                                nc.vector.tensor_tensor(
                                    out=an.rearrange("p (h c) -> p h c", h=H),
                                    in0=acc.rearrange("p (h c) -> p h c", h=H),
                                    in1=so[:].unsqueeze(2).broadcast_to([P, H, C]),
                                    op=mybir.AluOpType.mult)
                                nc.vector.tensor_tensor(out=an, in0=an, in1=red_out, op=mybir.AluOpType.add)
                                acc = an
                            else:
                                acc = red_out
                            j0 += jc

                    if "edge" in ablate or "epi" in ablate:
                        continue
                    # group-wide: o = sum_h acc / ((den + 1e-16) * H)
                    rcp = sb12.tile([P, GRP * H], F32, tag="rcpg")
                    nc.vector.tensor_scalar_add(rcp[:, :gs * H], deng[:, :gs * H], 1e-16)
                    nc.vector.reciprocal(rcp[:, :gs * H], rcp[:, :gs * H])
                    if H > 1:
                        nc.vector.tensor_scalar_mul(rcp[:, :gs * H], rcp[:, :gs * H], 1.0 / H)
                    nc.vector.tensor_tensor(
                        out=accg[:, :gs * HC].rearrange("p (g h c) -> p g h c", g=gs, h=H),
                        in0=accg[:, :gs * HC].rearrange("p (g h c) -> p g h c", g=gs, h=H),
                        in1=rcp[:, :gs * H].rearrange("p (g h) -> p g h", g=gs).unsqueeze(3)
                            .broadcast_to([P, gs, H, C]),
                        op=mybir.AluOpType.mult)
                    if H > 1:
                        og = sb6.tile([P, GRP * C], F32, tag="og")
                        a4 = accg[:, :gs * HC].rearrange("p (g h c) -> p g h c", g=gs, h=H)
                        nc.vector.tensor_tensor(out=og[:, :gs * C].rearrange("p (g c) -> p g c", g=gs),
                                                in0=a4[:, :, 0, :], in1=a4[:, :, 1, :],
                                                op=mybir.AluOpType.add)
                        for hh in range(2, H):
                            nc.vector.tensor_tensor(out=og[:, :gs * C].rearrange("p (g c) -> p g c", g=gs),
                                                    in0=og[:, :gs * C].rearrange("p (g c) -> p g c", g=gs),
                                                    in1=a4[:, :, hh, :],
                                                    op=mybir.AluOpType.add)
                        osrc = og
                    else:
                        osrc = accg
                    for gi in range(gs):
                        t = g0 + gi
                        ptr = psum4.tile([P, P], F32, tag="tr", space="PSUM")
                        nc.tensor.transpose(out=ptr[:], in_=osrc[:, gi * C:(gi + 1) * C], identity=ident[:])
                        nc.vector.tensor_copy(oT[:, t * P:(t + 1) * P], ptr[:])

                # ---- phase 4: batchnorm (+relu) ----
                nsum = sb4.tile([P, 1], F32, tag="nsum")
                nsq = sb4.tile([P, 1], F32, tag="nsq")
                nc.vector.tensor_reduce(out=nsum[:], in_=oT[:], axis=mybir.AxisListType.X,
                                        op=mybir.AluOpType.add)
                yTn = sb.tile([P, shard_pad], F32, tag="yT{}".format(l % 2))
                nc.scalar.activation(yTn[:], oT[:], mybir.ActivationFunctionType.Square,
                                     accum_out=nsq[:])
                ar_in = dram.tile([P, 2], F32, tag=f"arin{l}")
                ar_out = dram.tile([P, 2], F32, tag=f"arout{l}")
                st2 = sb4.tile([P, 2], F32, tag="st2")
                nc.vector.tensor_copy(st2[:, 0:1], nsum[:])
                nc.vector.tensor_copy(st2[:, 1:2], nsq[:])
                nc.gpsimd.dma_start(ar_in[:], st2[:])
                if "coll" not in ablate:
                    nc.gpsimd.collective_compute(
                        "AllReduce", mybir.AluOpType.add,
                        replica_groups=[list(range(NCORES))],
                        ins=[ar_in.opt()], outs=[ar_out.opt()],
                    )
                stg = sb4.tile([P, 2], F32, tag="stg")
                nc.sync.dma_start(stg[:], ar_out[:])
                ntotal = float(NCORES * g["shard"])
                mu = sb4.tile([P, 1], F32, tag="mu")
                nc.vector.tensor_scalar_mul(mu[:], stg[:, 0:1], 1.0 / ntotal)
                var = sb4.tile([P, 1], F32, tag="var")
                nc.vector.tensor_scalar_mul(var[:], stg[:, 1:2], 1.0 / ntotal)
                musq = sb4.tile([P, 1], F32, tag="musq")
                nc.vector.tensor_tensor(out=musq[:], in0=mu[:], in1=mu[:], op=mybir.AluOpType.mult)
                nc.vector.tensor_tensor(out=var[:], in0=var[:], in1=musq[:], op=mybir.AluOpType.subtract)
                rstd = sb4.tile([P, 1], F32, tag="rstd")
                nc.vector.tensor_scalar_add(var[:], var[:], EPS_BN)
                nc.scalar.activation(rstd[:], var[:], mybir.ActivationFunctionType.Sqrt)
                nc.vector.reciprocal(rstd[:], rstd[:])
                scale = sb4.tile([P, 1], F32, tag="scale")
                nc.vector.tensor_tensor(out=scale[:], in0=gb_t[:, 2 * l:2 * l + 1], in1=rstd[:],
                                        op=mybir.AluOpType.mult)
                shift = sb4.tile([P, 1], F32, tag="shift")
                nc.vector.tensor_tensor(out=shift[:], in0=mu[:], in1=scale[:], op=mybir.AluOpType.mult)
                nc.vector.tensor_tensor(out=shift[:], in0=gb_t[:, 2 * l + 1:2 * l + 2], in1=shift[:],
                                        op=mybir.AluOpType.subtract)
                func = (mybir.ActivationFunctionType.Relu if l < n_l - 1
                        else mybir.ActivationFunctionType.Identity)
                nc.scalar.activation(yTn[:], oT[:], func, bias=shift[:], scale=scale[:])
                npad = shard_pad - g["shard"]
                if npad > 0 and l < n_l - 1:
                    nc.vector.memset(yTn[:, g["shard"]:], 0.0)
                yT = yTn

            nc.sync.dma_start(out_t[:], yT[:])

    nc.compile()
    return nc


# ----------------------------------------------------------------------------
# entry point
# ----------------------------------------------------------------------------

def build_for_inputs(x, edge_index, params_list, ablate=(), nlayers=3):
    """Build (nc, in_maps) without running. params_list = [(W, a_src, a_dst, gamma, beta), ...]"""
    x = np.asarray(x, np.float32)
    N, in_dim = x.shape
    g = _prep(np.asarray(edge_index), N)
    params = params_list[:nlayers]
    layers = []
    for (W, asr, ads, gmm, bet) in params:
        H, C = asr.shape
        HC = H * C
        R = ((HC + H) * 4 + 31) // 32 * 8
        layers.append({"H": H, "C": C, "R": R, "hs_off": HC})
    nc = _build_program(g, layers, in_dim, ablate=ablate)
    wexts = []
    for (W, asr, ads, gmm, bet), L in zip(params, layers):
        H, C = L["H"], L["C"]
        w_s = np.einsum("khc,hc->kh", W.reshape(W.shape[0], H, C), asr)
        w_d = np.einsum("khc,hc->kh", W.reshape(W.shape[0], H, C), ads)
        wexts.append(np.concatenate([W, w_s, w_d], axis=1).astype(np.float32))
    gbm = np.zeros((P, 2 * len(layers)), np.float32)
    for l, (W, asr, ads, gmm, bet) in enumerate(params):
        gbm[:len(gmm), 2 * l] = gmm
        gbm[:len(bet), 2 * l + 1] = bet
    shard, shard_pad = g["shard"], g["shard_pad"]
    in_maps = []
    for c in range(NCORES):
        nodes = g["out_nodes"][c]
        xT_c = np.zeros((in_dim, shard_pad), np.float32)
        xT_c[:, :shard] = x[nodes].T
        spad = np.zeros((P, 1), np.float32)
        lastbase = (g["ntiles"] - 1) * P
        for p in range(P):
            if lastbase + p >= shard:
                spad[p, 0] = -1e30
        m = {"xT": xT_c, "idx": np.ascontiguousarray(g["idx"][c]), "gb": gbm, "spad": spad}
        for l, w in enumerate(wexts):
            m[f"wext{l}"] = w
        in_maps.append(m)
    return nc, in_maps, g, layers


def kernel(x, edge_index,
           W0, a_src0, a_dst0, b0, gamma0, beta0,
           W1, a_src1, a_dst1, b1, gamma1, beta1,
           W2, a_src2, a_dst2, b2, gamma2, beta2, _profile=None, _nlayers=3):
    x = np.asarray(x, np.float32)
    N, in_dim = x.shape
    g = _prep(np.asarray(edge_index), N)

    params = [(np.asarray(W0, np.float32), np.asarray(a_src0, np.float32), np.asarray(a_dst0, np.float32),
               np.asarray(gamma0, np.float32), np.asarray(beta0, np.float32)),
              (np.asarray(W1, np.float32), np.asarray(a_src1, np.float32), np.asarray(a_dst1, np.float32),
               np.asarray(gamma1, np.float32), np.asarray(beta1, np.float32)),
              (np.asarray(W2, np.float32), np.asarray(a_src2, np.float32), np.asarray(a_dst2, np.float32),
               np.asarray(gamma2, np.float32), np.asarray(beta2, np.float32))]

    params = params[:_nlayers]
    layers = []
    for (W, asr, ads, gmm, bet) in params:
        H, C = asr.shape
        HC = H * C
        R = ((HC + H) * 4 + 31) // 32 * 8  # row f32 elems, 32B-aligned
        layers.append({"H": H, "C": C, "R": R, "hs_off": HC})

    nc = _build_program(g, layers, in_dim)

    # per-layer extended weights [K, H*C + 2H] = [W | W@a_src^T per head | W@a_dst^T]
    wexts = []
    for (W, asr, ads, gmm, bet), L in zip(params, layers):
        H, C = L["H"], L["C"]
        w_s = np.einsum("khc,hc->kh", W.reshape(W.shape[0], H, C), asr)
        w_d = np.einsum("khc,hc->kh", W.reshape(W.shape[0], H, C), ads)
        wexts.append(np.concatenate([W, w_s, w_d], axis=1).astype(np.float32))

    gb = np.zeros((P, 2 * len(layers)), np.float32)
    for l, (W, asr, ads, gmm, bet) in enumerate(params):
        gb[:len(gmm), 2 * l] = gmm
        gb[:len(bet), 2 * l + 1] = bet

    shard, shard_pad = g["shard"], g["shard_pad"]
    in_maps = []
    for c in range(NCORES):
        nodes = g["out_nodes"][c]
        xT_c = np.zeros((in_dim, shard_pad), np.float32)
        xT_c[:, :shard] = x[nodes].T
        spad = np.zeros((P, 1), np.float32)
        lastbase = (g["ntiles"] - 1) * P
        for p in range(P):
            if lastbase + p >= shard:
                spad[p, 0] = -1e30
        m = {"xT": xT_c, "idx": np.ascontiguousarray(g["idx"][c]), "gb": gb, "spad": spad}
        for l, w in enumerate(wexts):
            m[f"wext{l}"] = w
        in_maps.append(m)

    if _profile is not None:
        _profile["nc"] = nc
        _profile["in_maps"] = in_maps
    res = bass_utils.run_bass_kernel_spmd(nc, in_maps, core_ids=list(range(NCORES)))

    C_out = layers[-1]["C"]
    out = np.empty((N, C_out), np.float32)
    for c in range(NCORES):
        yT = res.results[c]["out"]           # [P(feat), shard_pad]
        out[g["out_nodes"][c]] = yT[:C_out, :shard].T
    if _profile is not None:
        _profile["results"] = res
    return out

